# revision 1
# baseline (speedup 1.0000x reference)
"""MoE head (router top-2 + per-expert GELU FFN) on 8 TRN2 NeuronCores.

Strategy (sharding hint: expert parallel with top-k dispatch):
  - Host computes the (tiny) router in float64: logits = x @ Wr.T,
    top-2 experts per token, softmax-over-top2 gate weights.
  - Tokens are gathered per expert on the host ("all-to-all dispatch"),
    padded to a fixed capacity C, and each of the 8 cores runs ONE
    expert's FFN over its gathered tokens:
        y_tok = gate * (gelu(x_tok @ W_in[e].T) @ W_out[e])
  - Host scatter-adds the two expert contributions per token.

  Device kernel (SPMD, one program, per-core data):
    mm1: h^T[dhid, tok] = W_in^T-chunks.T @ x^T-chunks  (PE, accumulate K=512)
    gelu: ACT engine, PSUM -> SBUF (float32r)
    mm2: y[tok, dh] = h^T-chunks.T @ W_out-chunks       (PE, accumulate K=2048)
    gate: DVE per-partition scalar multiply, then DMA out.

  Matmuls run in float32r (TF32-like, ~13 mantissa bits). Measured on HW:
  f32r 258 ns vs bf16 220 ns vs fp32 1071 ns per [128x128]@[128x512]
  matmul, so f32r is 4.15x faster than fp32 at ~15x better accuracy than
  bf16. End-to-end error vs the fp32 reference: 1.6e-4 scale-relative
  absmax. Capacity is the exact max expert count (ragged final subtile),
  and the input DMA stream is fine-grained + first-use-ordered so the PE
  starts ~2 descriptors in. Per-kernel device time ~87 us (loop-slope
  measured) with the PE busy ~77 us of it; the 8-core sparse-dispatch
  f32r compute floor for this shape is ~77 us. Final config: exact cap,
  fine-DMA, PSUM ps1=5/ps2=3 (all 8 banks), ybuf=4.
"""

import os

import numpy as np

P = 128
DH = 512
DHID = 2048
NE = 8
TOPK = 2
KC1 = DH // P  # k-chunks for mm1
KC2 = DHID // P  # k-chunks for mm2
N_CORES = 8

DEFAULT_CAP = 1152  # tokens per expert; E[count] = T*K/NE = 1024 for T=4096

_prog_cache: dict[tuple, object] = {}
LAST_EXEC_NS = None  # filled when MOE_TRACE=1
LAST_RESULTS = None
LAST_CAP = None


def _dtype_mode() -> str:
    return os.environ.get("MOE_DTYPE", "f32r")  # f32r | bf16 | fp32


def _groups_of(c: int) -> list[int]:
    """Token groups: each in [256, 512] so the f32r matmul moving dim stays
    >= 256 (4x slower below). All groups except the last are multiples of
    128, so every group offset is 128-aligned; the last group absorbs any
    remainder (its final mm2 subtile is partial-M)."""
    rem = c % 128
    if rem:
        last = 256 + rem  # in (256, 384)
        body = c - last
    else:
        last = None
        body = c
    gs = []
    left = body
    while left > 640:
        gs.append(512)
        left -= 512
    if left > 512:  # 640+128k leftovers: split into two >=256 chunks
        hi = (left // 256) * 128
        gs.extend([left - hi, hi])
    elif left:
        gs.append(left)
    if last is not None:
        gs.append(last)
    return gs


def _build_program(
    cap: int, mode: str, reps: int = 1, loop_n: int = 0, loop_scope: str = "all"
):
    """loop_n > 0 wraps the body in a runtime For_i loop (timing only).
    loop_scope="compute" hoists the DMA+round stage out of the loop."""
    import contextlib

    import concourse.mybir as mybir
    import concourse.tile as tile
    from concourse import bacc

    f32 = mybir.dt.float32
    mm_dt = {
        "f32r": mybir.dt.float32r,
        "bf16": mybir.dt.bfloat16,
        "fp32": mybir.dt.float32,
    }[mode]

    nc = bacc.Bacc(None, target_bir_lowering=False, debug=False)
    # Inputs are declared as float32r (same 4-byte layout as fp32; numpy side
    # is np.float32). The PE truncates fp32 -> f32r internally, so raw fp32
    # bytes DMA'd straight in give the same result as an explicit rounding
    # pass -- verified on HW -- and the whole stage/round stage disappears.
    in_dt = mybir.dt.float32r if mode == "f32r" else f32
    xg = nc.declare_dram_parameter("xg", [DH, cap], in_dt, isOutput=False)
    w_in_t = nc.declare_dram_parameter("w_in_t", [DH, DHID], in_dt, isOutput=False)
    w_out = nc.declare_dram_parameter("w_out", [DHID, DH], in_dt, isOutput=False)
    gcols = -(-cap // P)
    gate = nc.declare_dram_parameter("gate", [gcols * P], f32, isOutput=False)
    y = nc.declare_dram_parameter("y", [cap, DH], f32, isOutput=True)

    groups = _groups_of(cap)
    gelu = mybir.ActivationFunctionType.Gelu

    with tile.TileContext(nc) as tc:
        with (
            tc.tile_pool(name="persist", bufs=1) as persist,
            tc.tile_pool(name="stage", bufs=3) as stage,
            # hbuf=1 for very large caps (all-tokens-to-one-expert fallback)
            # so the SBUF budget still closes; slower but correct.
            tc.tile_pool(
                name="hbuf",
                bufs=int(os.environ.get("MOE_HBUF", "2" if cap <= 2304 else "1")),
            ) as hbuf,
            tc.tile_pool(name="ybuf", bufs=int(os.environ.get("MOE_YBUF", "4"))) as ybuf,
            tc.tile_pool(name="ps1", bufs=int(os.environ.get("MOE_PS1", "5")), space="PSUM") as ps1,
            tc.tile_pool(name="ps2", bufs=int(os.environ.get("MOE_PS2", "3")), space="PSUM") as ps2,
        ):
            state = {}

            def emit_loads():
                # ---- direct DMA into matmul-dtype tiles (no staging) ----
                if mode == "bf16":
                    # bf16 needs an on-device cast: stage fp32 then DVE copy.
                    xg_r = persist.tile([P, KC1, cap], mm_dt, tag="xg_r")
                    for kc in range(KC1):
                        stg = stage.tile([P, cap], f32, tag="stg_x")
                        nc.sync.dma_start(out=stg, in_=xg[kc * P : (kc + 1) * P, :])
                        nc.vector.tensor_copy(xg_r[:, kc, :], stg)
                    w_in_r = persist.tile([P, KC1, DHID], mm_dt, tag="w_in_r")
                    for kc in range(KC1):
                        stg = stage.tile([P, DHID], f32, tag="stg_wi")
                        nc.sync.dma_start(
                            out=stg, in_=w_in_t[kc * P : (kc + 1) * P, :]
                        )
                        nc.vector.tensor_copy(w_in_r[:, kc, :], stg)
                    w_out_r = persist.tile([P, KC2, DH], mm_dt, tag="w_out_r")
                    for kc in range(KC2):
                        stg = stage.tile([P, DH], f32, tag="stg_wo")
                        nc.sync.dma_start(
                            out=stg, in_=w_out[kc * P : (kc + 1) * P, :]
                        )
                        nc.vector.tensor_copy(w_out_r[:, kc, :], stg)
                elif os.environ.get("MOE_FINEDMA", "1") == "1":
                    # First-use-ordered fine-grained input stream: the PE's
                    # first work (mm1 group0, d0-3) needs only xg[:,g0] and
                    # w_in quarter q0 -- land those ~2MB first so the PE
                    # starts ~6us in instead of waiting on whole chunks.
                    xg_r = persist.tile([P, KC1, cap], in_dt, tag="xg_r")
                    w_in_r = persist.tile([P, KC1, DHID], in_dt, tag="w_in_r")
                    w_out_r = persist.tile([P, KC2, DH], in_dt, tag="w_out_r")
                    Q = DHID // 4
                    g_offs = []
                    o = 0
                    for gsz in groups:
                        g_offs.append((o, gsz))
                        o += gsz

                    def dma_xg(kc, gi):
                        o, gsz = g_offs[gi]
                        nc.sync.dma_start(
                            out=xg_r[:, kc, o : o + gsz],
                            in_=xg[kc * P : (kc + 1) * P, o : o + gsz],
                        )

                    def dma_win(kc, q):
                        nc.sync.dma_start(
                            out=w_in_r[:, kc, q * Q : (q + 1) * Q],
                            in_=w_in_t[kc * P : (kc + 1) * P, q * Q : (q + 1) * Q],
                        )

                    if os.environ.get("MOE_HEADI", "1") == "1":
                        for kc in range(KC1):
                            dma_xg(kc, 0)
                            dma_win(kc, 0)
                        for kc in range(KC1):
                            dma_win(kc, 1)
                            dma_xg(kc, 1)
                    else:
                        for kc in range(KC1):
                            dma_xg(kc, 0)
                        for kc in range(KC1):
                            dma_win(kc, 0)
                        for kc in range(KC1):
                            dma_win(kc, 1)
                        for kc in range(KC1):
                            dma_xg(kc, 1)
                    for kc in range(KC1):
                        dma_win(kc, 2)
                    for kc in range(KC1):
                        dma_win(kc, 3)
                    for kc2 in range(KC2 // 2):
                        nc.sync.dma_start(
                            out=w_out_r[:, kc2 * 2 : (kc2 + 1) * 2, :],
                            in_=w_out.rearrange("(kc p) d -> p kc d", p=P)[
                                :, kc2 * 2 : (kc2 + 1) * 2, :
                            ],
                        )
                    for gi in range(2, len(groups)):
                        for kc in range(KC1):
                            dma_xg(kc, gi)
                else:
                    xg_r = persist.tile([P, KC1, cap], in_dt, tag="xg_r")
                    for kc in range(KC1):
                        nc.sync.dma_start(
                            out=xg_r[:, kc, :], in_=xg[kc * P : (kc + 1) * P, :]
                        )
                    w_in_r = persist.tile([P, KC1, DHID], in_dt, tag="w_in_r")
                    for kc in range(KC1):
                        for h2 in range(2):  # halves for earlier first-use
                            nc.sync.dma_start(
                                out=w_in_r[:, kc, h2 * (DHID // 2) : (h2 + 1) * (DHID // 2)],
                                in_=w_in_t[
                                    kc * P : (kc + 1) * P,
                                    h2 * (DHID // 2) : (h2 + 1) * (DHID // 2),
                                ],
                            )
                    w_out_r = persist.tile([P, KC2, DH], in_dt, tag="w_out_r")
                    for kc4 in range(KC2 // 4):
                        nc.sync.dma_start(
                            out=w_out_r[:, kc4 * 4 : (kc4 + 1) * 4, :],
                            in_=w_out.rearrange("(kc p) d -> p kc d", p=P)[
                                :, kc4 * 4 : (kc4 + 1) * 4, :
                            ],
                        )
                gate_sb = persist.tile([P, gcols], f32, tag="gate_sb")
                nc.sync.dma_start(
                    out=gate_sb, in_=gate.rearrange("(g p) -> p g", p=P)
                )
                state.update(
                    xg_r=xg_r, w_in_r=w_in_r, w_out_r=w_out_r, gate_sb=gate_sb
                )

            def emit_compute():
                # ---- mm1 -> gelu -> mm2 -> gate -> out, per token-group ----
                # Phase order is staggered (mm1 g0, mm1 g1, mm2 g0, mm1 g2,
                # mm2 g1, mm2 g2) so the first mm2 starts ~2 mm1-phases into
                # the kernel, giving the 4MB w_out DMA stream time to land
                # without stalling the PE. Needs 2 live h tiles (hbuf=2).
                xg_r, w_in_r = state["xg_r"], state["w_in_r"]
                w_out_r, gate_sb = state["w_out_r"], state["gate_sb"]
                offs = []
                off = 0
                for gsz in groups:
                    offs.append(off)
                    off += gsz
                h_tiles = {}

                def mm1_phase(gi):
                    gsz, off = groups[gi], offs[gi]
                    h_r = hbuf.tile([P, KC2, gsz], mm_dt, tag="h_r")
                    h_tiles[gi] = h_r
                    for d in range(KC2):
                        ps = ps1.tile([P, gsz], f32, tag="p1")
                        for kc in range(KC1):
                            nc.tensor.matmul(
                                ps,
                                w_in_r[:, kc, d * P : (d + 1) * P],
                                xg_r[:, kc, off : off + gsz],
                                start=(kc == 0),
                                stop=(kc == KC1 - 1),
                            )
                        nc.scalar.activation(h_r[:, d, :], ps, gelu)

                def mm2_phase(gi):
                    gsz, off = groups[gi], offs[gi]
                    h_r = h_tiles.pop(gi)
                    for s in range(-(-gsz // P)):
                        m = min(P, gsz - s * P)  # last subtile may be partial
                        pt = ps2.tile([P, DH], f32, tag="p2")
                        for d in range(KC2):
                            nc.tensor.matmul(
                                pt[:m, :],
                                h_r[:, d, s * P : s * P + m],
                                w_out_r[:, d, :],
                                start=(d == 0),
                                stop=(d == KC2 - 1),
                            )
                        tok0 = off + s * P
                        y_sb = ybuf.tile([P, DH], f32, tag="y_sb")
                        nc.vector.tensor_scalar_mul(
                            y_sb[:m, :], pt[:m, :], gate_sb[:m, tok0 // P : tok0 // P + 1]
                        )
                        nc.sync.dma_start(out=y[tok0 : tok0 + m, :], in_=y_sb[:m, :])

                n_g = len(groups)
                if n_g == 1 or os.environ.get("MOE_STAGGER", "1") != "1":
                    for gi in range(n_g):
                        mm1_phase(gi)
                        mm2_phase(gi)
                else:
                    mm1_phase(0)
                    mm1_phase(1)
                    for gi in range(2, n_g):
                        mm2_phase(gi - 2)
                        mm1_phase(gi)
                    mm2_phase(n_g - 2)
                    mm2_phase(n_g - 1)

            if loop_n and loop_scope == "compute":
                emit_loads()
                with tc.For_i(0, loop_n, 1):
                    for _rep in range(reps):
                        emit_compute()
            elif loop_n:
                with tc.For_i(0, loop_n, 1):
                    for _rep in range(reps):
                        emit_loads()
                        emit_compute()
            else:
                for _rep in range(reps):
                    emit_loads()
                    emit_compute()

    nc.compile()
    return nc


def _get_program(cap: int, mode: str):
    key = (cap, mode)
    if key not in _prog_cache:
        _prog_cache[key] = _build_program(cap, mode)
    return _prog_cache[key]


def kernel(x, Wr, W_in, W_out):
    global LAST_EXEC_NS, LAST_RESULTS
    from concourse.bass_utils import run_bass_kernel_spmd

    x = np.ascontiguousarray(np.asarray(x), dtype=np.float32)
    Wr = np.asarray(Wr, dtype=np.float32)
    W_in = np.asarray(W_in, dtype=np.float32)
    W_out = np.asarray(W_out, dtype=np.float32)
    T = x.shape[0]

    # ---- host router (fp64: strictly more accurate than the fp32 ref) ----
    logits = x.astype(np.float64) @ Wr.astype(np.float64).T  # (T, NE)
    part = np.argpartition(-logits, TOPK - 1, axis=1)[:, :TOPK]
    vals = np.take_along_axis(logits, part, axis=1)
    order = np.argsort(-vals, axis=1, kind="stable")
    idx = np.take_along_axis(part, order, axis=1)  # (T, 2) desc
    ar = np.arange(T)
    v1 = logits[ar, idx[:, 0]]
    v2 = logits[ar, idx[:, 1]]
    e2 = np.exp(v2 - v1)
    w1 = (1.0 / (1.0 + e2)).astype(np.float32)
    w2 = (e2 / (1.0 + e2)).astype(np.float32)

    tok_lists, gate_lists = [], []
    for e in range(NE):
        s1 = np.nonzero(idx[:, 0] == e)[0]
        s2 = np.nonzero(idx[:, 1] == e)[0]
        tok_lists.append(np.concatenate([s1, s2]))
        gate_lists.append(np.concatenate([w1[s1], w2[s2]]))
    max_count = max(len(t) for t in tok_lists)

    cap_env = os.environ.get("MOE_CAP")
    cap = int(cap_env) if cap_env else -(-max_count // 4) * 4  # exact-ish
    if max_count > cap:
        cap = -(-max_count // 4) * 4
    cap = max(cap, 384)
    mode = _dtype_mode()
    global LAST_CAP
    LAST_CAP = cap
    nc = _get_program(cap, mode)

    in_maps = []
    for e in range(NE):
        toks = tok_lists[e]
        xg = np.zeros((DH, cap), np.float32)
        xg[:, : len(toks)] = x[toks].T
        g = np.zeros((-(-cap // P) * P,), np.float32)
        g[: len(toks)] = gate_lists[e]
        in_maps.append(
            {
                "xg": xg,
                "w_in_t": np.ascontiguousarray(W_in[e].T),
                "w_out": np.ascontiguousarray(W_out[e]),
                "gate": g,
            }
        )

    trace = os.environ.get("MOE_TRACE", "0") == "1"
    res = run_bass_kernel_spmd(
        nc,
        in_maps,
        list(range(N_CORES)),
        trace=trace,
        trace_cores=list(range(N_CORES)) if trace else None,
    )
    LAST_EXEC_NS = res.exec_time_ns
    LAST_RESULTS = res

    out = np.zeros((T, DH), np.float32)
    for e in range(NE):
        toks = tok_lists[e]
        if len(toks):
            out[toks] += res.results[e]["y"][: len(toks)]
    return out

